# revision 20
# baseline (speedup 1.0000x reference)
"""Trainium2 Bass kernel for a single DecoderRNN step (LSTM cell + soft-dot
attention + vocab projection), SPMD over 8 NeuronCores.

Sharding:
- LSTM cell: tensor-parallel over the hidden dim (each core computes gate
  columns for its H/8 slice, full batch, M=128 matmuls), then an AllGather
  of transposed h1 chunks lands directly in the [K=H, M=B] matmul layout.
- Attention: data-parallel over batch (16 rows/core); scores on the Vector
  engine (fused multiply-reduce), softmax denominators via an all-ones
  matmul, weighted sums as block-diagonal PE matmuls.
- weighted is AllGather'd, W_out runs full-batch on every core, and the
  vocab projection is tensor-parallel (4000 cols/core, host-assembled).

Host only slices / transposes / casts. Outputs match the reference:
(h_1 [128,1024], c_1 [128,1024], alpha [128,512], logit [128,32000]).
"""

import contextlib
import os
import sys
import types

for _p in ("/opt/trn_rl_repo", "/root/.axon_site/_ro/trn_rl_repo"):
    if os.path.isdir(_p) and _p not in sys.path:
        sys.path.append(_p)

import numpy as np

import concourse.bass as bass
import concourse.mybir as mybir
import concourse.tile as tile
from concourse import bacc
from concourse.bass_utils import run_bass_kernel_spmd
from concourse.masks import make_identity


# NTFF profiling hook shim (used only when BASS_TRACE=1); missing in image.
def _install_ntff_shim():
    if "antenv.axon_hooks" in sys.modules:
        return
    try:
        import antenv  # noqa: F401
        from trn_agent_boot.trn_boot import _ntff_profile_via_ctypes
    except Exception:
        return
    so = "/opt/axon/libaxon_pjrt.so"
    hook = _ntff_profile_via_ctypes(so) if os.path.exists(so) else None
    mod = types.ModuleType("antenv.axon_hooks")
    mod.get_axon_ntff_profile_hook = lambda: hook
    mod.set_axon_ntff_profile_hook = lambda h: None
    sys.modules["antenv.axon_hooks"] = mod
    sys.modules["antenv"].axon_hooks = mod


_install_ntff_shim()

V, E, H, B, S = 32000, 512, 1024, 128, 512
NC = 8
BC = B // NC        # 16 batch rows per core
HC = H // NC        # 128 hidden cols per core (TP-LSTM slice)
VC = V // NC        # 4000 vocab cols per core
NEG = -1e9
P = 128
G4 = 4 * H          # 4096 gate units
NV = 8              # vocab n-splits per core
NVW = VC // NV      # 500 cols per split
NKI = E // P        # 4 K-chunks from x
NKH = H // P        # 8 K-chunks from h0
NSC = S // P        # 4 s-chunks

F32 = mybir.dt.float32
# fp32r runs matmul rows 4x faster but at ~tf32 precision (~1e-3 end-to-end
# rel err measured on HW); fp32 measures ~1.4e-5. Default to exact.
MM_DT = mybir.dt.float32r if os.environ.get("KMM", "f32") == "f32r" else F32

DEBUG = bool(int(os.environ.get("KDEBUG", "0")))
# Dev bisect knob: 1=LSTM only, 2=+attention, 3=+wout, 4=full (default)
STAGE = int(os.environ.get("KSTAGE", "4"))


class _StageStop(Exception):
    pass


def _f32(ap):
    """View an MM_DT AP as plain fp32 for vector/scalar-engine access."""
    if MM_DT is F32:
        return ap
    return ap.bitcast(F32)


def _build():
    nc = bacc.Bacc("TRN2", target_bir_lowering=False, debug=False, num_devices=NC)

    # ---- I/O (per-core values supplied by the host shard loop) ----
    idx = nc.dram_tensor("idx", [B, 1], mybir.dt.int32, kind="ExternalInput")
    rowsel = nc.dram_tensor("rowsel", [P, BC], mybir.dt.int32, kind="ExternalInput")
    emb = nc.dram_tensor("emb", [V, E], F32, kind="ExternalInput")
    h0T = nc.dram_tensor("h0T", [H, B], MM_DT, kind="ExternalInput")
    c0c = nc.dram_tensor("c0c", [B, HC], F32, kind="ExternalInput")
    ctxd = nc.dram_tensor("ctxd", [BC, S, H], MM_DT, kind="ExternalInput")
    mnegT = nc.dram_tensor("mnegT", [S, BC], F32, kind="ExternalInput")
    # TP gate weights for this core's 4x128 column strips, grouped for DMA:
    # [3 groups, 4 chunks, 128, 512]
    wg_tp = nc.dram_tensor("wg_tp", [3, 4, P, 512], MM_DT, kind="ExternalInput")
    bihc = nc.dram_tensor("bihc", [1, 512], F32, kind="ExternalInput")
    bhhc = nc.dram_tensor("bhhc", [1, 512], F32, kind="ExternalInput")
    win_blk = nc.dram_tensor("win_blk", [2, NKH, P, 512], MM_DT, kind="ExternalInput")
    wout_blk = nc.dram_tensor("wout_blk", [2, 2 * NKH, P, 512], MM_DT,
                              kind="ExternalInput")
    wdec_blk = nc.dram_tensor("wdec_blk", [NV, NKH, P, NVW], MM_DT,
                              kind="ExternalInput")
    bdec = nc.dram_tensor("bdec", [1, VC], F32, kind="ExternalInput")

    h1o = nc.dram_tensor("h1o", [B, HC], F32, kind="ExternalOutput")
    c1o = nc.dram_tensor("c1o", [B, HC], F32, kind="ExternalOutput")
    alphao = nc.dram_tensor("alphao", [BC, S], F32, kind="ExternalOutput")
    logito = nc.dram_tensor("logito", [B, VC], F32, kind="ExternalOutput")
    dbg = {}
    if DEBUG:
        dbg["x"] = nc.dram_tensor("dbg_x", [B, E], F32, kind="ExternalOutput")
        dbg["target"] = nc.dram_tensor("dbg_target", [B, H], F32, kind="ExternalOutput")
        dbg["ht"] = nc.dram_tensor("dbg_ht", [B, H], F32, kind="ExternalOutput")

    with tile.TileContext(nc) as tc, contextlib.ExitStack() as ctx:
        pp = ctx.enter_context(tc.tile_pool(name="persist", bufs=1))
        wstream = ctx.enter_context(tc.tile_pool(name="wstream", bufs=3))
        ctxpool = ctx.enter_context(tc.tile_pool(name="ctxpool", bufs=3))
        small = ctx.enter_context(tc.tile_pool(name="small", bufs=2))
        psum = ctx.enter_context(tc.tile_pool(name="psum", bufs=1, space="PSUM"))
        dram = ctx.enter_context(tc.tile_pool(name="dram", bufs=1, space="DRAM"))

        def ptile(shape, name, dtype=F32):
            return pp.tile(shape, dtype, tag=name, name=name)

        try:
            # ---- warm-up sync: tiny AllGather absorbs inter-core launch
            # skew while DMA prefetch runs; the real collectives then start
            # aligned and cost ~13us instead of ~45us.
            warm_sb = pp.tile([1, 16], F32, tag="warm_sb", name="warm_sb")
            nc.gpsimd.memset(warm_sb[:], 0.0)
            warmd = dram.tile([1, 16], F32, tag="warmd", name="warmd")
            warmg = dram.tile([NC, 16], F32, addr_space="Shared", tag="warmg",
                              name="warmg")
            nc.sync.dma_start(warmd[:], warm_sb[:])
            nc.gpsimd.collective_compute(
                "AllGather",
                mybir.AluOpType.bypass,
                replica_groups=[list(range(NC))],
                ins=[warmd.opt()],
                outs=[warmg.opt()],
            )

            # ---- constants ----
            ident = ptile([P, P], "ident")
            make_identity(nc, ident[:])
            ones_sq = ptile([P, P], "ones_sq")
            nc.gpsimd.memset(ones_sq[:], 1.0)

            # ---- embedding gather (all 128 rows on every core) ----
            idx_sb = ptile([B, 1], "idx_sb", dtype=mybir.dt.int32)
            nc.sync.dma_start(idx_sb[:], idx[:])
            x_sb = ptile([B, E], "x_sb")
            nc.gpsimd.indirect_dma_start(
                out=x_sb[:],
                out_offset=None,
                in_=emb[:],
                in_offset=bass.IndirectOffsetOnAxis(ap=idx_sb[:, :1], axis=0),
            )
            if DEBUG:
                nc.sync.dma_start(dbg["x"][:], x_sb[:])

            # ---- x^T (E on partitions) ----
            xT = ptile([P, NKI, B], "xT", dtype=MM_DT)
            for k in range(NKI):
                pt = psum.tile([P, P], F32, tag="tr", bufs=2, name="pt_x")
                nc.tensor.transpose(pt[:], x_sb[:, k * P:(k + 1) * P], ident[:])
                nc.vector.tensor_copy(xT[:, k, :], pt[:])

            # ---- h0^T load ([H, B] dram -> [128, 8, 128]) ----
            h0T_sb = ptile([P, NKH, B], "h0T_sb", dtype=MM_DT)
            nc.sync.dma_start(h0T_sb[:], h0T[:].rearrange("(o p) b -> p o b", p=P))

            # ---- bias strips (b_ih + b_hh) via accumulate DMA ----
            bsumd = dram.tile([1, 512], F32, tag="bsumd", name="bsumd")
            nc.gpsimd.dma_start(bsumd[:], bihc[:])
            nc.gpsimd.dma_start(bsumd[:], bhhc[:], accum_op=mybir.AluOpType.add)

            # ---- TP-LSTM gates: [B, 512] = x @ Wih_c^T + h0 @ Whh_c^T + b ----
            # columns = this core's [i|f|g|o] strips of 128 each
            NKG = NKI + NKH
            pg = psum.tile([B, 512], F32, tag="acc", bufs=2, name="pg")
            for kg in range(3):
                wt = wstream.tile([P, 4, 512], MM_DT, tag="w", name="wt_g")
                nc.sync.dma_start(wt[:], wg_tp[kg].rearrange("k p j -> p k j"))
                for kk in range(4):
                    k = kg * 4 + kk
                    lhsT = xT[:, k, :] if k < NKI else h0T_sb[:, k - NKI, :]
                    nc.tensor.matmul(
                        pg[:], lhsT, wt[:, kk, :],
                        start=(k == 0), stop=(k == NKG - 1),
                    )
            bsl = small.tile([1, 512], F32, tag="bs", name="bsl")
            nc.sync.dma_start(bsl[:], bsumd[:])
            bb = small.tile([B, 512], F32, tag="bb", name="bbt")
            nc.gpsimd.partition_broadcast(bb[:], bsl[:])
            gb = small.tile([B, 512], F32, tag="gb", name="gbt")
            nc.vector.tensor_add(gb[:], pg[:], bb[:])
            func_by_gate = [
                mybir.ActivationFunctionType.Sigmoid,  # i
                mybir.ActivationFunctionType.Sigmoid,  # f
                mybir.ActivationFunctionType.Tanh,     # g
                mybir.ActivationFunctionType.Sigmoid,  # o
            ]
            gact = [ptile([B, HC], f"gate{gi}") for gi in range(4)]
            for gi in range(4):
                nc.scalar.activation(
                    gact[gi][:], gb[:, gi * HC:(gi + 1) * HC], func_by_gate[gi]
                )

            # c1 = f*c0 + i*g ; h1 = o*tanh(c1)   (this core's H-slice)
            i_sb, f_sb, g_sb, o_sb = gact
            c0_sb = ptile([B, HC], "c0_sb")
            nc.sync.dma_start(c0_sb[:], c0c[:])
            c1_sb = ptile([B, HC], "c1_sb")
            th_sb = ptile([B, HC], "th_sb")
            nc.vector.tensor_mul(c1_sb[:], f_sb[:], c0_sb[:])
            nc.vector.tensor_mul(th_sb[:], i_sb[:], g_sb[:])
            nc.vector.tensor_add(c1_sb[:], c1_sb[:], th_sb[:])
            nc.scalar.activation(th_sb[:], c1_sb[:], mybir.ActivationFunctionType.Tanh)
            h1_sb = ptile([B, HC], "h1_sb")
            nc.vector.tensor_mul(h1_sb[:], o_sb[:], th_sb[:])
            nc.sync.dma_start(h1o[:], h1_sb[:])
            nc.sync.dma_start(c1o[:], c1_sb[:])

            if STAGE < 2:
                raise _StageStop

            # ---- AllGather h1^T: every core contributes its [128h, 128b] ----
            pt = psum.tile([P, P], F32, tag="tr", bufs=2, name="pt_h1")
            nc.tensor.transpose(pt[:], h1_sb[:], ident[:])
            h1tc = ptile([P, P], "h1tc", dtype=MM_DT)
            nc.vector.tensor_copy(h1tc[:], pt[:])
            h1td = dram.tile([P, P], MM_DT, tag="h1td", name="h1td")
            h1tg = dram.tile([H, B], MM_DT, addr_space="Shared", tag="h1tg",
                             name="h1tg")
            nc.sync.dma_start(h1td[:], h1tc[:])
            nc.gpsimd.collective_compute(
                "AllGather",
                mybir.AluOpType.bypass,
                replica_groups=[list(range(NC))],
                ins=[h1td.opt()],
                outs=[h1tg.opt()],
            )
            h1T = ptile([P, NKH, B], "h1T", dtype=MM_DT)
            nc.sync.dma_start(h1T[:], h1tg[:].rearrange("(o p) b -> p o b", p=P))

            # ---- target = h1 @ W_in^T (full batch), then gather own rows ----
            target_sb = ptile([B, H], "target_sb")
            for n2 in range(2):
                cols = slice(n2 * 512, (n2 + 1) * 512)
                ptg = psum.tile([B, 512], F32, tag="acc", bufs=2, name="ptg")
                for kg in range(NKH // 4):
                    wt = wstream.tile([P, 4, 512], MM_DT, tag="w", name="wt_t")
                    nc.sync.dma_start(
                        wt[:], win_blk[n2, kg * 4:(kg + 1) * 4].rearrange("k p j -> p k j"))
                    for kk in range(4):
                        k = kg * 4 + kk
                        nc.tensor.matmul(
                            ptg[:], h1T[:, k, :], wt[:, kk, :],
                            start=(k == 0), stop=(k == NKH - 1),
                        )
                nc.scalar.copy(target_sb[:, cols], ptg[:])
            if DEBUG:
                nc.sync.dma_start(dbg["target"][:], target_sb[:])
            # target rows reach the score ops via fused gather+broadcast:
            # column b of rowsel repeats global row id 16c+b on all 128
            # partitions, so one indirect gather replicates that target row
            # across the whole partition dim.
            targetd = dram.tile([B, H], F32, tag="targetd", name="targetd")
            nc.sync.dma_start(targetd[:], target_sb[:])
            rsel_sb = ptile([P, BC], "rsel_sb", dtype=mybir.dt.int32)
            nc.sync.dma_start(rsel_sb[:], rowsel[:])

            # ---- masked scores + softmax + weighted sum, per batch row ----
            mneg_sb = ptile([P, NSC, BC], "mneg_sb")
            nc.sync.dma_start(mneg_sb[:], mnegT[:].rearrange("(o p) b -> p o b", p=P))

            alpha_cols = ptile([P, NSC, BC], "alpha_cols")   # alpha, s on partitions
            nc.gpsimd.memset(alpha_cols[:], 0.0)
            zeros_ab = ptile([P, NSC, BC], "zeros_ab")
            nc.gpsimd.memset(zeros_ab[:], 0.0)
            weighted_sb = ptile([BC, H], "weighted_sb")
            junk = small.tile([P, H], F32, tag="junk", bufs=1, name="junk")
            # one shared PSUM accumulator: each b contributes only its own row
            # (its alpha tile is zero in every other column), so after the loop
            # all BC rows are valid and a single aligned copy suffices.
            pw = psum.tile([BC, H], F32, tag="w", bufs=1, name="pwt")

            for b in range(BC):
                # target row b replicated to all partitions in one gather
                tbt = small.tile([P, H], F32, tag="tb", bufs=3, name="tbt")
                nc.gpsimd.indirect_dma_start(
                    out=tbt[:],
                    out_offset=None,
                    in_=targetd[:],
                    in_offset=bass.IndirectOffsetOnAxis(ap=rsel_sb[:, b:b + 1], axis=0),
                )
                tb = tbt[:]
                # ctx rows for batch b: one DMA, [128, 4, 1024] (4KB runs)
                ct = ctxpool.tile([P, NSC, H], MM_DT, tag="ctx", name="ct")
                nc.sync.dma_start(
                    ct[:], ctxd[b].rearrange("(c p) h -> p c h", p=P)
                )
                # scores[s] = sum_h ctx[s,h] * target[h]  (+ mask bias)
                sc = small.tile([P, NSC], F32, tag="sc", bufs=4, name="sct")
                for c in range(NSC):
                    # NB: InstTensorTensorReduce faults at runtime on this
                    # stack; InstTensorScalarPtr(is_scalar_tensor_tensor) with
                    # accum_out is the working fused multiply-reduce.
                    nc.vector.scalar_tensor_tensor(
                        out=junk[:],
                        in0=_f32(ct[:, c, :]),
                        scalar=0.0,
                        in1=tb,
                        op0=mybir.AluOpType.add,
                        op1=mybir.AluOpType.mult,
                        accum_out=sc[:, c:c + 1],
                    )
                nc.vector.tensor_add(sc[:], sc[:], mneg_sb[:, :, b])
                # softmax over all 512 entries (s on partitions x 4 chunks).
                # No max-subtraction: |scores| << 80 for this model family, so
                # exp/sum in fp32 matches the reference to ~1e-7 relative.
                pe_sb = small.tile([P, NSC], F32, tag="pe", bufs=4, name="pet")
                nc.scalar.activation(pe_sb[:], sc[:], mybir.ActivationFunctionType.Exp)
                # partition sums broadcast back to every partition: ones^T @ pe
                psm = psum.tile([P, NSC], F32, tag="sm", bufs=1, name="psm")
                nc.tensor.matmul(psm[:], ones_sq[:], pe_sb[:], start=True, stop=True)
                tot = small.tile([P, 1], F32, tag="tot", bufs=4, name="tott")
                nc.vector.tensor_reduce(
                    tot[:], psm[:], axis=mybir.AxisListType.X, op=mybir.AluOpType.add
                )
                rinv = small.tile([P, 1], F32, tag="rinv", bufs=4, name="rinvt")
                nc.vector.reciprocal(rinv[:], tot[:])
                nc.vector.tensor_scalar_mul(alpha_cols[:, :, b], pe_sb[:], rinv[:])
                # fresh zeroed alpha tile with only column b set -> block-diag
                ab = small.tile([P, NSC, BC], MM_DT, tag="ab", bufs=3, name="abt")
                nc.vector.tensor_copy(_f32(ab[:]), zeros_ab[:])  # keep GPSIMD free
                nc.vector.tensor_scalar_mul(ab[:, :, b], pe_sb[:], rinv[:])
                # weighted[b] = sum_s alpha[s] * ctx[s, :]
                for c in range(NSC):
                    for nh in range(2):
                        nc.tensor.matmul(
                            pw[:, nh * 512:(nh + 1) * 512],
                            ab[:, c, :],
                            ct[:, c, nh * 512:(nh + 1) * 512],
                            start=(b == 0 and c == 0),
                            stop=(b == BC - 1 and c == NSC - 1),
                            skip_group_check=True,
                        )
            nc.scalar.copy(weighted_sb[:], pw[:])

            # ---- alpha output (transpose alpha_cols -> [BC, S]) ----
            alphaT = ptile([BC, S], "alphaT")
            for c in range(NSC):
                pt = psum.tile([P, P], F32, tag="tr", bufs=2, name="pt_a")
                nc.tensor.transpose(pt[:BC, :], alpha_cols[:, c, :], ident[:])
                nc.vector.tensor_copy(alphaT[:, c * P:(c + 1) * P], pt[:BC, :])
            nc.sync.dma_start(alphao[:], alphaT[:])

            if STAGE < 3:
                raise _StageStop

            # ---- AllGather weighted -> full batch on every core ----
            wgtd = dram.tile([BC, H], F32, tag="wgtd", name="wgtd")
            wgtg = dram.tile([B, H], F32, addr_space="Shared", tag="wgtg",
                             name="wgtg")
            nc.sync.dma_start(wgtd[:], weighted_sb[:])
            nc.gpsimd.collective_compute(
                "AllGather",
                mybir.AluOpType.bypass,
                replica_groups=[list(range(NC))],
                ins=[wgtd.opt()],
                outs=[wgtg.opt()],
            )
            wgt_full = ptile([B, H], "wgt_full")
            nc.sync.dma_start(wgt_full[:], wgtg[:])
            wgtT = ptile([P, NKH, B], "wgtT", dtype=MM_DT)
            for k in range(NKH):
                pt = psum.tile([P, P], F32, tag="tr", bufs=2, name="pt_w")
                nc.tensor.transpose(pt[:], wgt_full[:, k * P:(k + 1) * P], ident[:])
                nc.vector.tensor_copy(wgtT[:, k, :], pt[:])

            # ---- h_tilde = tanh([weighted, h1] @ W_out^T), full batch ----
            # h1 half (K-chunks 8..15) is emitted first so it accumulates
            # while the weighted AllGather is still in flight.
            ht_sb = ptile([B, H], "ht_sb")
            for n2 in range(2):
                cols = slice(n2 * 512, (n2 + 1) * 512)
                ptg = psum.tile([B, 512], F32, tag="acc", bufs=2, name="pto")
                for half in (1, 0):   # 1: h1 chunks 8-15 first, 0: weighted
                    for kg in range(2):
                        kbase = half * NKH + kg * 4
                        wt = wstream.tile([P, 4, 512], MM_DT, tag="w", name="wt_o")
                        nc.sync.dma_start(
                            wt[:],
                            wout_blk[n2, kbase:kbase + 4].rearrange("k p j -> p k j"))
                        for kk in range(4):
                            k = kbase + kk
                            lhsT = wgtT[:, k, :] if half == 0 else h1T[:, k - NKH, :]
                            nc.tensor.matmul(
                                ptg[:], lhsT, wt[:, kk, :],
                                start=(half == 1 and kg == 0 and kk == 0),
                                stop=(half == 0 and kg == 1 and kk == 3),
                            )
                nc.scalar.activation(
                    ht_sb[:, cols], ptg[:], mybir.ActivationFunctionType.Tanh
                )
            if DEBUG:
                nc.sync.dma_start(dbg["ht"][:], ht_sb[:])

            if STAGE < 4:
                raise _StageStop

            # ---- h_tilde^T ----
            htT = ptile([P, NKH, B], "htT", dtype=MM_DT)
            for k in range(NKH):
                pt = psum.tile([P, P], F32, tag="tr", bufs=2, name="pt_ht")
                nc.tensor.transpose(pt[:], ht_sb[:, k * P:(k + 1) * P], ident[:])
                nc.vector.tensor_copy(htT[:, k, :], pt[:])

            # ---- logits shard: [B, VC] = h_tilde @ W_dec^T + b_dec ----
            for nv in range(NV):
                cols = slice(nv * NVW, (nv + 1) * NVW)
                pl = psum.tile([B, NVW], F32, tag="pl", bufs=1, name="pl")
                for kg in range(NKH // 4):
                    wt = wstream.tile([P, 4, NVW], MM_DT, tag="wd", bufs=3, name="wt_d")
                    nc.sync.dma_start(
                        wt[:], wdec_blk[nv, kg * 4:(kg + 1) * 4].rearrange("k p j -> p k j"))
                    for kk in range(4):
                        k = kg * 4 + kk
                        nc.tensor.matmul(
                            pl[:], htT[:, k, :], wt[:, kk, :],
                            start=(k == 0), stop=(k == NKH - 1),
                        )
                bsl = small.tile([1, NVW], F32, tag="bs", name="bsld")
                nc.sync.dma_start(bsl[:], bdec[:, cols])
                bb = small.tile([B, NVW], F32, tag="bb", name="bbd")
                nc.gpsimd.partition_broadcast(bb[:], bsl[:])
                lg = small.tile([B, NVW], F32, tag="gb", name="lgt")
                nc.vector.tensor_add(lg[:], pl[:], bb[:])
                nc.sync.dma_start(logito[:, cols], lg[:])
        except _StageStop:
            pass

    nc.compile()
    return nc


_CACHE = {}
last_results = None


def _get_program():
    key = (MM_DT, DEBUG, STAGE)
    if key not in _CACHE:
        _CACHE[key] = _build()
    return _CACHE[key]


def kernel(previous_word, h_0, c_0, ctx, ctx_mask, emb, W_ih, W_hh, b_ih, b_hh,
           W_in, W_out, W_dec, b_dec):
    global last_results
    f32 = np.float32
    previous_word = np.asarray(previous_word)
    idx_all = np.ascontiguousarray(previous_word.reshape(B, 1).astype(np.int32))
    h_0 = np.asarray(h_0, dtype=f32)
    c_0 = np.asarray(c_0, dtype=f32)
    ctx = np.asarray(ctx, dtype=f32)
    mask_neg = np.where(np.asarray(ctx_mask), f32(NEG), f32(0.0)).astype(f32)
    emb = np.ascontiguousarray(np.asarray(emb, dtype=f32))
    W_ih = np.asarray(W_ih, dtype=f32)
    W_hh = np.asarray(W_hh, dtype=f32)
    b_ih = np.asarray(b_ih, dtype=f32).reshape(G4)
    b_hh = np.asarray(b_hh, dtype=f32).reshape(G4)
    win_blk = np.ascontiguousarray(
        np.asarray(W_in, dtype=f32).T.reshape(NKH, P, 2, 512).transpose(2, 0, 1, 3))
    wout_blk = np.ascontiguousarray(
        np.asarray(W_out, dtype=f32).T.reshape(2 * NKH, P, 2, 512).transpose(2, 0, 1, 3))
    W_dec = np.asarray(W_dec, dtype=f32)
    b_dec = np.asarray(b_dec, dtype=f32)
    h0T_full = np.ascontiguousarray(h_0.T)

    nc = _get_program()

    in_maps = []
    for c in range(NC):
        rows = slice(c * BC, (c + 1) * BC)
        vs = slice(c * VC, (c + 1) * VC)
        hcols = slice(c * HC, (c + 1) * HC)
        # this core's gate column strips within [i|f|g|o]
        strips = np.concatenate([np.arange(g * H + c * HC, g * H + (c + 1) * HC)
                                 for g in range(4)])
        wg_tp = np.ascontiguousarray(
            np.concatenate([
                W_ih.T[:, strips].reshape(NKI, P, 512),
                W_hh.T[:, strips].reshape(NKH, P, 512),
            ]).reshape(3, 4, P, 512))
        wdec_blk = np.ascontiguousarray(
            W_dec[vs].T.reshape(NKH, P, NV, NVW).transpose(2, 0, 1, 3))
        in_maps.append({
            "idx": idx_all,
            "rowsel": np.ascontiguousarray(
                np.tile(np.arange(c * BC, (c + 1) * BC, dtype=np.int32), (P, 1))),
            "emb": emb,
            "h0T": h0T_full,
            "c0c": np.ascontiguousarray(c_0[:, hcols]),
            "ctxd": np.ascontiguousarray(ctx[rows]),
            "mnegT": np.ascontiguousarray(mask_neg[rows].T),
            "wg_tp": wg_tp,
            "bihc": np.ascontiguousarray(b_ih[strips].reshape(1, 512)),
            "bhhc": np.ascontiguousarray(b_hh[strips].reshape(1, 512)),
            "win_blk": win_blk,
            "wout_blk": wout_blk,
            "wdec_blk": wdec_blk,
            "bdec": np.ascontiguousarray(b_dec[vs].reshape(1, VC)),
        })

    res = run_bass_kernel_spmd(nc, in_maps, list(range(NC)))
    last_results = res
    r = res.results
    h_1 = np.concatenate([r[c]["h1o"] for c in range(NC)], axis=1)
    c_1 = np.concatenate([r[c]["c1o"] for c in range(NC)], axis=1)
    alpha = np.concatenate([r[c]["alphao"] for c in range(NC)], axis=0)
    logit = np.concatenate([r[c]["logito"] for c in range(NC)], axis=1)
    return h_1, c_1, alpha, logit


# revision 22
# speedup vs baseline: 1.0879x; 1.0879x over previous
"""Trainium2 Bass kernel for a single DecoderRNN step (LSTM cell + soft-dot
attention + vocab projection), SPMD over 8 NeuronCores.

Sharding:
- LSTM cell: tensor-parallel over the hidden dim (each core computes gate
  columns for its H/8 slice, full batch, M=128 matmuls), then an AllGather
  of transposed h1 chunks lands directly in the [K=H, M=B] matmul layout.
- Attention: data-parallel over batch (16 rows/core); scores on the Vector
  engine (fused multiply-reduce), softmax denominators via an all-ones
  matmul, weighted sums as block-diagonal PE matmuls.
- weighted is AllGather'd, W_out runs full-batch on every core, and the
  vocab projection is tensor-parallel (4000 cols/core, host-assembled).

Host only slices / transposes / casts. Outputs match the reference:
(h_1 [128,1024], c_1 [128,1024], alpha [128,512], logit [128,32000]).
"""

import contextlib
import os
import sys
import types

for _p in ("/opt/trn_rl_repo", "/root/.axon_site/_ro/trn_rl_repo"):
    if os.path.isdir(_p) and _p not in sys.path:
        sys.path.append(_p)

import numpy as np

import concourse.bass as bass
import concourse.mybir as mybir
import concourse.tile as tile
from concourse import bacc
from concourse.bass_utils import run_bass_kernel_spmd
from concourse.masks import make_identity


# NTFF profiling hook shim (used only when BASS_TRACE=1); missing in image.
def _install_ntff_shim():
    if "antenv.axon_hooks" in sys.modules:
        return
    try:
        import antenv  # noqa: F401
        from trn_agent_boot.trn_boot import _ntff_profile_via_ctypes
    except Exception:
        return
    so = "/opt/axon/libaxon_pjrt.so"
    hook = _ntff_profile_via_ctypes(so) if os.path.exists(so) else None
    mod = types.ModuleType("antenv.axon_hooks")
    mod.get_axon_ntff_profile_hook = lambda: hook
    mod.set_axon_ntff_profile_hook = lambda h: None
    sys.modules["antenv.axon_hooks"] = mod
    sys.modules["antenv"].axon_hooks = mod


_install_ntff_shim()

V, E, H, B, S = 32000, 512, 1024, 128, 512
NC = 8
BC = B // NC        # 16 batch rows per core
HC = H // NC        # 128 hidden cols per core (TP-LSTM slice)
VC = V // NC        # 4000 vocab cols per core
NEG = -1e9
P = 128
G4 = 4 * H          # 4096 gate units
NV = 8              # vocab n-splits per core
NVW = VC // NV      # 500 cols per split
NKI = E // P        # 4 K-chunks from x
NKH = H // P        # 8 K-chunks from h0
NSC = S // P        # 4 s-chunks

F32 = mybir.dt.float32
# fp32r runs matmul rows 4x faster but at ~tf32 precision (~1e-3 end-to-end
# rel err measured on HW); fp32 measures ~1.4e-5. Default to exact.
MM_DT = mybir.dt.float32r if os.environ.get("KMM", "f32") == "f32r" else F32

DEBUG = bool(int(os.environ.get("KDEBUG", "0")))
# Dev bisect knob: 1=LSTM only, 2=+attention, 3=+wout, 4=full (default)
STAGE = int(os.environ.get("KSTAGE", "4"))


class _StageStop(Exception):
    pass


def _f32(ap):
    """View an MM_DT AP as plain fp32 for vector/scalar-engine access."""
    if MM_DT is F32:
        return ap
    return ap.bitcast(F32)


def _build():
    nc = bacc.Bacc("TRN2", target_bir_lowering=False, debug=False, num_devices=NC)

    # ---- I/O (per-core values supplied by the host shard loop) ----
    idx = nc.dram_tensor("idx", [B, 1], mybir.dt.int32, kind="ExternalInput")
    rowsel = nc.dram_tensor("rowsel", [BC, 1], mybir.dt.int32, kind="ExternalInput")
    emb = nc.dram_tensor("emb", [V, E], F32, kind="ExternalInput")
    h0T = nc.dram_tensor("h0T", [H, B], MM_DT, kind="ExternalInput")
    c0c = nc.dram_tensor("c0c", [B, HC], F32, kind="ExternalInput")
    ctxd = nc.dram_tensor("ctxd", [BC, S, H], MM_DT, kind="ExternalInput")
    mnegT = nc.dram_tensor("mnegT", [S, BC], F32, kind="ExternalInput")
    # TP gate weights for this core's 4x128 column strips, grouped for DMA:
    # [3 groups, 4 chunks, 128, 512]
    wg_tp = nc.dram_tensor("wg_tp", [3, 4, P, 512], MM_DT, kind="ExternalInput")
    bihc = nc.dram_tensor("bihc", [1, 512], F32, kind="ExternalInput")
    bhhc = nc.dram_tensor("bhhc", [1, 512], F32, kind="ExternalInput")
    win_blk = nc.dram_tensor("win_blk", [2, NKH, P, 512], MM_DT, kind="ExternalInput")
    wout_blk = nc.dram_tensor("wout_blk", [2, 2 * NKH, P, 512], MM_DT,
                              kind="ExternalInput")
    wdec_blk = nc.dram_tensor("wdec_blk", [NV, NKH, P, NVW], MM_DT,
                              kind="ExternalInput")
    bdec = nc.dram_tensor("bdec", [1, VC], F32, kind="ExternalInput")

    h1o = nc.dram_tensor("h1o", [B, HC], F32, kind="ExternalOutput")
    c1o = nc.dram_tensor("c1o", [B, HC], F32, kind="ExternalOutput")
    alphao = nc.dram_tensor("alphao", [BC, S], F32, kind="ExternalOutput")
    logito = nc.dram_tensor("logito", [B, VC], F32, kind="ExternalOutput")
    dbg = {}
    if DEBUG:
        dbg["x"] = nc.dram_tensor("dbg_x", [B, E], F32, kind="ExternalOutput")
        dbg["target"] = nc.dram_tensor("dbg_target", [B, H], F32, kind="ExternalOutput")
        dbg["ht"] = nc.dram_tensor("dbg_ht", [B, H], F32, kind="ExternalOutput")

    with tile.TileContext(nc) as tc, contextlib.ExitStack() as ctx:
        pp = ctx.enter_context(tc.tile_pool(name="persist", bufs=1))
        wstream = ctx.enter_context(tc.tile_pool(name="wstream", bufs=3))
        ctxpool = ctx.enter_context(tc.tile_pool(name="ctxpool", bufs=3))
        small = ctx.enter_context(tc.tile_pool(name="small", bufs=2))
        psum = ctx.enter_context(tc.tile_pool(name="psum", bufs=1, space="PSUM"))
        dram = ctx.enter_context(tc.tile_pool(name="dram", bufs=1, space="DRAM"))

        def ptile(shape, name, dtype=F32):
            return pp.tile(shape, dtype, tag=name, name=name)

        try:
            # ---- warm-up sync: tiny AllGather absorbs inter-core launch
            # skew while DMA prefetch runs; the real collectives then start
            # aligned and cost ~13us instead of ~45us.
            warm_sb = pp.tile([1, 16], F32, tag="warm_sb", name="warm_sb")
            nc.gpsimd.memset(warm_sb[:], 0.0)
            warmd = dram.tile([1, 16], F32, tag="warmd", name="warmd")
            warmg = dram.tile([NC, 16], F32, addr_space="Shared", tag="warmg",
                              name="warmg")
            nc.sync.dma_start(warmd[:], warm_sb[:])
            nc.gpsimd.collective_compute(
                "AllGather",
                mybir.AluOpType.bypass,
                replica_groups=[list(range(NC))],
                ins=[warmd.opt()],
                outs=[warmg.opt()],
            )

            # ---- constants ----
            ident = ptile([P, P], "ident")
            make_identity(nc, ident[:])
            ones_sq = ptile([P, P], "ones_sq")
            nc.gpsimd.memset(ones_sq[:], 1.0)

            # ---- embedding gather (all 128 rows on every core) ----
            idx_sb = ptile([B, 1], "idx_sb", dtype=mybir.dt.int32)
            nc.sync.dma_start(idx_sb[:], idx[:])
            x_sb = ptile([B, E], "x_sb")
            nc.gpsimd.indirect_dma_start(
                out=x_sb[:],
                out_offset=None,
                in_=emb[:],
                in_offset=bass.IndirectOffsetOnAxis(ap=idx_sb[:, :1], axis=0),
            )
            if DEBUG:
                nc.sync.dma_start(dbg["x"][:], x_sb[:])

            # ---- x^T (E on partitions) ----
            xT = ptile([P, NKI, B], "xT", dtype=MM_DT)
            for k in range(NKI):
                pt = psum.tile([P, P], F32, tag="tr", bufs=2, name="pt_x")
                nc.tensor.transpose(pt[:], x_sb[:, k * P:(k + 1) * P], ident[:])
                nc.vector.tensor_copy(xT[:, k, :], pt[:])

            # ---- h0^T load ([H, B] dram -> [128, 8, 128]) ----
            h0T_sb = ptile([P, NKH, B], "h0T_sb", dtype=MM_DT)
            nc.sync.dma_start(h0T_sb[:], h0T[:].rearrange("(o p) b -> p o b", p=P))

            # ---- bias strips (b_ih + b_hh) via accumulate DMA ----
            bsumd = dram.tile([1, 512], F32, tag="bsumd", name="bsumd")
            nc.gpsimd.dma_start(bsumd[:], bihc[:])
            nc.gpsimd.dma_start(bsumd[:], bhhc[:], accum_op=mybir.AluOpType.add)

            # ---- TP-LSTM gates: [B, 512] = x @ Wih_c^T + h0 @ Whh_c^T + b ----
            # columns = this core's [i|f|g|o] strips of 128 each
            NKG = NKI + NKH
            pg = psum.tile([B, 512], F32, tag="acc", bufs=1, name="pg")
            for kg in range(3):
                wt = wstream.tile([P, 4, 512], MM_DT, tag="w", name="wt_g")
                nc.sync.dma_start(wt[:], wg_tp[kg].rearrange("k p j -> p k j"))
                for kk in range(4):
                    k = kg * 4 + kk
                    lhsT = xT[:, k, :] if k < NKI else h0T_sb[:, k - NKI, :]
                    nc.tensor.matmul(
                        pg[:], lhsT, wt[:, kk, :],
                        start=(k == 0), stop=(k == NKG - 1),
                    )
            bsl = small.tile([1, 512], F32, tag="bs", name="bsl")
            nc.sync.dma_start(bsl[:], bsumd[:])
            bb = small.tile([B, 512], F32, tag="bb", name="bbt")
            nc.gpsimd.partition_broadcast(bb[:], bsl[:])
            gb = small.tile([B, 512], F32, tag="gb", name="gbt")
            nc.vector.tensor_add(gb[:], pg[:], bb[:])
            func_by_gate = [
                mybir.ActivationFunctionType.Sigmoid,  # i
                mybir.ActivationFunctionType.Sigmoid,  # f
                mybir.ActivationFunctionType.Tanh,     # g
                mybir.ActivationFunctionType.Sigmoid,  # o
            ]
            gact = [ptile([B, HC], f"gate{gi}") for gi in range(4)]
            for gi in range(4):
                nc.scalar.activation(
                    gact[gi][:], gb[:, gi * HC:(gi + 1) * HC], func_by_gate[gi]
                )

            # c1 = f*c0 + i*g ; h1 = o*tanh(c1)   (this core's H-slice)
            i_sb, f_sb, g_sb, o_sb = gact
            c0_sb = ptile([B, HC], "c0_sb")
            nc.sync.dma_start(c0_sb[:], c0c[:])
            c1_sb = ptile([B, HC], "c1_sb")
            th_sb = ptile([B, HC], "th_sb")
            nc.vector.tensor_mul(c1_sb[:], f_sb[:], c0_sb[:])
            nc.vector.tensor_mul(th_sb[:], i_sb[:], g_sb[:])
            nc.vector.tensor_add(c1_sb[:], c1_sb[:], th_sb[:])
            nc.scalar.activation(th_sb[:], c1_sb[:], mybir.ActivationFunctionType.Tanh)
            h1_sb = ptile([B, HC], "h1_sb")
            nc.vector.tensor_mul(h1_sb[:], o_sb[:], th_sb[:])
            nc.sync.dma_start(h1o[:], h1_sb[:])
            nc.sync.dma_start(c1o[:], c1_sb[:])

            if STAGE < 2:
                raise _StageStop

            # ---- AllGather h1^T: every core contributes its [128h, 128b] ----
            pt = psum.tile([P, P], F32, tag="tr", bufs=2, name="pt_h1")
            nc.tensor.transpose(pt[:], h1_sb[:], ident[:])
            h1tc = ptile([P, P], "h1tc", dtype=MM_DT)
            nc.vector.tensor_copy(h1tc[:], pt[:])
            h1td = dram.tile([P, P], MM_DT, tag="h1td", name="h1td")
            h1tg = dram.tile([H, B], MM_DT, addr_space="Shared", tag="h1tg",
                             name="h1tg")
            nc.sync.dma_start(h1td[:], h1tc[:])
            nc.gpsimd.collective_compute(
                "AllGather",
                mybir.AluOpType.bypass,
                replica_groups=[list(range(NC))],
                ins=[h1td.opt()],
                outs=[h1tg.opt()],
            )
            h1T = ptile([P, NKH, B], "h1T", dtype=MM_DT)
            nc.sync.dma_start(h1T[:], h1tg[:].rearrange("(o p) b -> p o b", p=P))

            # ---- target = h1 @ W_in^T (full batch), then gather own rows ----
            target_sb = ptile([B, H], "target_sb")
            for n2 in range(2):
                cols = slice(n2 * 512, (n2 + 1) * 512)
                ptg = psum.tile([B, 512], F32, tag="acc", bufs=1, name="ptg")
                for kg in range(NKH // 4):
                    wt = wstream.tile([P, 4, 512], MM_DT, tag="w", name="wt_t")
                    nc.sync.dma_start(
                        wt[:], win_blk[n2, kg * 4:(kg + 1) * 4].rearrange("k p j -> p k j"))
                    for kk in range(4):
                        k = kg * 4 + kk
                        nc.tensor.matmul(
                            ptg[:], h1T[:, k, :], wt[:, kk, :],
                            start=(k == 0), stop=(k == NKH - 1),
                        )
                nc.scalar.copy(target_sb[:, cols], ptg[:])
            if DEBUG:
                nc.sync.dma_start(dbg["target"][:], target_sb[:])
            # own 16 rows via indirect gather (row ids are a per-core input)
            targetd = dram.tile([B, H], F32, tag="targetd", name="targetd")
            nc.sync.dma_start(targetd[:], target_sb[:])
            rsel_sb = ptile([BC, 1], "rsel_sb", dtype=mybir.dt.int32)
            nc.sync.dma_start(rsel_sb[:], rowsel[:])
            target_own = ptile([BC, H], "target_own")
            nc.gpsimd.indirect_dma_start(
                out=target_own[:],
                out_offset=None,
                in_=targetd[:],
                in_offset=bass.IndirectOffsetOnAxis(ap=rsel_sb[:, :1], axis=0),
            )

            # ---- masked scores + softmax + weighted sum, per batch row ----
            mneg_sb = ptile([P, NSC, BC], "mneg_sb")
            nc.sync.dma_start(mneg_sb[:], mnegT[:].rearrange("(o p) b -> p o b", p=P))

            alpha_cols = ptile([P, NSC, BC], "alpha_cols")   # alpha, s on partitions
            nc.gpsimd.memset(alpha_cols[:], 0.0)
            zeros_ab = ptile([P, NSC, BC], "zeros_ab")
            nc.gpsimd.memset(zeros_ab[:], 0.0)
            weighted_sb = ptile([BC, H], "weighted_sb")
            junk = small.tile([P, H], F32, tag="junk", bufs=1, name="junk")
            # one shared PSUM accumulator: each b contributes only its own row
            # (its alpha tile is zero in every other column), so after the loop
            # all BC rows are valid and a single aligned copy suffices.
            pw = psum.tile([BC, H], F32, tag="w", bufs=1, name="pwt")

            for b in range(BC):
                # broadcast target rows across partitions, two rows per
                # GPSIMD op (partition_broadcast wants partition-0 input)
                if b % 2 == 0:
                    tb0 = small.tile([1, 2 * H], F32, tag="tb0", bufs=2, name="tb0t")
                    nc.sync.dma_start(tb0[:, :H], target_own[b:b + 1, :])
                    nc.sync.dma_start(tb0[:, H:], target_own[b + 1:b + 2, :])
                    tbp = small.tile([P, 2 * H], F32, tag="tb", bufs=2, name="tbt")
                    nc.gpsimd.partition_broadcast(tbp[:], tb0[:])
                tb = tbp[:, (b % 2) * H:(b % 2 + 1) * H]
                # ctx rows for batch b: one DMA, [128, 4, 1024] (4KB runs)
                ct = ctxpool.tile([P, NSC, H], MM_DT, tag="ctx", name="ct")
                nc.sync.dma_start(
                    ct[:], ctxd[b].rearrange("(c p) h -> p c h", p=P)
                )
                # scores[s] = sum_h ctx[s,h] * target[h]  (+ mask bias)
                sc = small.tile([P, NSC], F32, tag="sc", bufs=4, name="sct")
                for c in range(NSC):
                    # NB: InstTensorTensorReduce faults at runtime on this
                    # stack; InstTensorScalarPtr(is_scalar_tensor_tensor) with
                    # accum_out is the working fused multiply-reduce.
                    nc.vector.scalar_tensor_tensor(
                        out=junk[:],
                        in0=_f32(ct[:, c, :]),
                        scalar=0.0,
                        in1=tb,
                        op0=mybir.AluOpType.add,
                        op1=mybir.AluOpType.mult,
                        accum_out=sc[:, c:c + 1],
                    )
                nc.vector.tensor_add(sc[:], sc[:], mneg_sb[:, :, b])
                # softmax over all 512 entries (s on partitions x 4 chunks).
                # No max-subtraction: |scores| << 80 for this model family, so
                # exp/sum in fp32 matches the reference to ~1e-7 relative.
                pe_sb = small.tile([P, NSC], F32, tag="pe", bufs=4, name="pet")
                nc.scalar.activation(pe_sb[:], sc[:], mybir.ActivationFunctionType.Exp)
                # partition sums broadcast back to every partition: ones^T @ pe
                psm = psum.tile([P, NSC], F32, tag="sm", bufs=2, name="psm")
                nc.tensor.matmul(psm[:], ones_sq[:], pe_sb[:], start=True, stop=True)
                tot = small.tile([P, 1], F32, tag="tot", bufs=4, name="tott")
                nc.vector.tensor_reduce(
                    tot[:], psm[:], axis=mybir.AxisListType.X, op=mybir.AluOpType.add
                )
                rinv = small.tile([P, 1], F32, tag="rinv", bufs=4, name="rinvt")
                nc.vector.reciprocal(rinv[:], tot[:])
                nc.vector.tensor_scalar_mul(alpha_cols[:, :, b], pe_sb[:], rinv[:])
                # fresh zeroed alpha tile with only column b set -> block-diag
                ab = small.tile([P, NSC, BC], MM_DT, tag="ab", bufs=3, name="abt")
                nc.vector.tensor_copy(_f32(ab[:]), zeros_ab[:])  # keep GPSIMD free
                nc.vector.tensor_scalar_mul(ab[:, :, b], pe_sb[:], rinv[:])
                # weighted[b] = sum_s alpha[s] * ctx[s, :]
                for c in range(NSC):
                    for nh in range(2):
                        nc.tensor.matmul(
                            pw[:, nh * 512:(nh + 1) * 512],
                            ab[:, c, :],
                            ct[:, c, nh * 512:(nh + 1) * 512],
                            start=(b == 0 and c == 0),
                            stop=(b == BC - 1 and c == NSC - 1),
                            skip_group_check=True,
                        )
            nc.scalar.copy(weighted_sb[:], pw[:])

            # ---- alpha output (transpose alpha_cols -> [BC, S]) ----
            alphaT = ptile([BC, S], "alphaT")
            for c in range(NSC):
                pt = psum.tile([P, P], F32, tag="tr", bufs=2, name="pt_a")
                nc.tensor.transpose(pt[:BC, :], alpha_cols[:, c, :], ident[:])
                nc.vector.tensor_copy(alphaT[:, c * P:(c + 1) * P], pt[:BC, :])
            nc.sync.dma_start(alphao[:], alphaT[:])

            if STAGE < 3:
                raise _StageStop

            # ---- AllGather weighted -> full batch on every core ----
            wgtd = dram.tile([BC, H], F32, tag="wgtd", name="wgtd")
            wgtg = dram.tile([B, H], F32, addr_space="Shared", tag="wgtg",
                             name="wgtg")
            nc.sync.dma_start(wgtd[:], weighted_sb[:])
            nc.gpsimd.collective_compute(
                "AllGather",
                mybir.AluOpType.bypass,
                replica_groups=[list(range(NC))],
                ins=[wgtd.opt()],
                outs=[wgtg.opt()],
            )
            wgt_full = ptile([B, H], "wgt_full")
            nc.sync.dma_start(wgt_full[:], wgtg[:])
            wgtT = ptile([P, NKH, B], "wgtT", dtype=MM_DT)
            for k in range(NKH):
                pt = psum.tile([P, P], F32, tag="tr", bufs=2, name="pt_w")
                nc.tensor.transpose(pt[:], wgt_full[:, k * P:(k + 1) * P], ident[:])
                nc.vector.tensor_copy(wgtT[:, k, :], pt[:])

            # ---- h_tilde = tanh([weighted, h1] @ W_out^T), full batch ----
            # h1 half (K-chunks 8..15) is emitted first so it accumulates
            # while the weighted AllGather is still in flight.
            ht_sb = ptile([B, H], "ht_sb")
            for n2 in range(2):
                cols = slice(n2 * 512, (n2 + 1) * 512)
                ptg = psum.tile([B, 512], F32, tag="acc", bufs=1, name="pto")
                for half in (1, 0):   # 1: h1 chunks 8-15 first, 0: weighted
                    for kg in range(2):
                        kbase = half * NKH + kg * 4
                        wt = wstream.tile([P, 4, 512], MM_DT, tag="w", name="wt_o")
                        nc.sync.dma_start(
                            wt[:],
                            wout_blk[n2, kbase:kbase + 4].rearrange("k p j -> p k j"))
                        for kk in range(4):
                            k = kbase + kk
                            lhsT = wgtT[:, k, :] if half == 0 else h1T[:, k - NKH, :]
                            nc.tensor.matmul(
                                ptg[:], lhsT, wt[:, kk, :],
                                start=(half == 1 and kg == 0 and kk == 0),
                                stop=(half == 0 and kg == 1 and kk == 3),
                            )
                nc.scalar.activation(
                    ht_sb[:, cols], ptg[:], mybir.ActivationFunctionType.Tanh
                )
            if DEBUG:
                nc.sync.dma_start(dbg["ht"][:], ht_sb[:])

            if STAGE < 4:
                raise _StageStop

            # ---- h_tilde^T ----
            htT = ptile([P, NKH, B], "htT", dtype=MM_DT)
            for k in range(NKH):
                pt = psum.tile([P, P], F32, tag="tr", bufs=2, name="pt_ht")
                nc.tensor.transpose(pt[:], ht_sb[:, k * P:(k + 1) * P], ident[:])
                nc.vector.tensor_copy(htT[:, k, :], pt[:])

            # ---- logits shard: [B, VC] = h_tilde @ W_dec^T + b_dec ----
            for nv in range(NV):
                cols = slice(nv * NVW, (nv + 1) * NVW)
                pl = psum.tile([B, NVW], F32, tag="pl", bufs=1, name="pl")
                for kg in range(NKH // 4):
                    wt = wstream.tile([P, 4, NVW], MM_DT, tag="wd", bufs=3, name="wt_d")
                    nc.sync.dma_start(
                        wt[:], wdec_blk[nv, kg * 4:(kg + 1) * 4].rearrange("k p j -> p k j"))
                    for kk in range(4):
                        k = kg * 4 + kk
                        nc.tensor.matmul(
                            pl[:], htT[:, k, :], wt[:, kk, :],
                            start=(k == 0), stop=(k == NKH - 1),
                        )
                bsl = small.tile([1, NVW], F32, tag="bs", name="bsld")
                nc.sync.dma_start(bsl[:], bdec[:, cols])
                bb = small.tile([B, NVW], F32, tag="bb", name="bbd")
                nc.gpsimd.partition_broadcast(bb[:], bsl[:])
                lg = small.tile([B, NVW], F32, tag="gb", name="lgt")
                nc.vector.tensor_add(lg[:], pl[:], bb[:])
                nc.sync.dma_start(logito[:, cols], lg[:])
        except _StageStop:
            pass

    nc.compile()
    return nc


_CACHE = {}
last_results = None


def _get_program():
    key = (MM_DT, DEBUG, STAGE)
    if key not in _CACHE:
        _CACHE[key] = _build()
    return _CACHE[key]


def kernel(previous_word, h_0, c_0, ctx, ctx_mask, emb, W_ih, W_hh, b_ih, b_hh,
           W_in, W_out, W_dec, b_dec):
    global last_results
    f32 = np.float32
    previous_word = np.asarray(previous_word)
    idx_all = np.ascontiguousarray(previous_word.reshape(B, 1).astype(np.int32))
    h_0 = np.asarray(h_0, dtype=f32)
    c_0 = np.asarray(c_0, dtype=f32)
    ctx = np.asarray(ctx, dtype=f32)
    mask_neg = np.where(np.asarray(ctx_mask), f32(NEG), f32(0.0)).astype(f32)
    emb = np.ascontiguousarray(np.asarray(emb, dtype=f32))
    W_ih = np.asarray(W_ih, dtype=f32)
    W_hh = np.asarray(W_hh, dtype=f32)
    b_ih = np.asarray(b_ih, dtype=f32).reshape(G4)
    b_hh = np.asarray(b_hh, dtype=f32).reshape(G4)
    win_blk = np.ascontiguousarray(
        np.asarray(W_in, dtype=f32).T.reshape(NKH, P, 2, 512).transpose(2, 0, 1, 3))
    wout_blk = np.ascontiguousarray(
        np.asarray(W_out, dtype=f32).T.reshape(2 * NKH, P, 2, 512).transpose(2, 0, 1, 3))
    W_dec = np.asarray(W_dec, dtype=f32)
    b_dec = np.asarray(b_dec, dtype=f32)
    h0T_full = np.ascontiguousarray(h_0.T)

    nc = _get_program()

    in_maps = []
    for c in range(NC):
        rows = slice(c * BC, (c + 1) * BC)
        vs = slice(c * VC, (c + 1) * VC)
        hcols = slice(c * HC, (c + 1) * HC)
        # this core's gate column strips within [i|f|g|o]
        strips = np.concatenate([np.arange(g * H + c * HC, g * H + (c + 1) * HC)
                                 for g in range(4)])
        wg_tp = np.ascontiguousarray(
            np.concatenate([
                W_ih.T[:, strips].reshape(NKI, P, 512),
                W_hh.T[:, strips].reshape(NKH, P, 512),
            ]).reshape(3, 4, P, 512))
        wdec_blk = np.ascontiguousarray(
            W_dec[vs].T.reshape(NKH, P, NV, NVW).transpose(2, 0, 1, 3))
        in_maps.append({
            "idx": idx_all,
            "rowsel": np.arange(c * BC, (c + 1) * BC, dtype=np.int32).reshape(BC, 1),
            "emb": emb,
            "h0T": h0T_full,
            "c0c": np.ascontiguousarray(c_0[:, hcols]),
            "ctxd": np.ascontiguousarray(ctx[rows]),
            "mnegT": np.ascontiguousarray(mask_neg[rows].T),
            "wg_tp": wg_tp,
            "bihc": np.ascontiguousarray(b_ih[strips].reshape(1, 512)),
            "bhhc": np.ascontiguousarray(b_hh[strips].reshape(1, 512)),
            "win_blk": win_blk,
            "wout_blk": wout_blk,
            "wdec_blk": wdec_blk,
            "bdec": np.ascontiguousarray(b_dec[vs].reshape(1, VC)),
        })

    res = run_bass_kernel_spmd(nc, in_maps, list(range(NC)))
    last_results = res
    r = res.results
    h_1 = np.concatenate([r[c]["h1o"] for c in range(NC)], axis=1)
    c_1 = np.concatenate([r[c]["c1o"] for c in range(NC)], axis=1)
    alpha = np.concatenate([r[c]["alphao"] for c in range(NC)], axis=0)
    logit = np.concatenate([r[c]["logito"] for c in range(NC)], axis=1)
    return h_1, c_1, alpha, logit
